# revision 15
# baseline (speedup 1.0000x reference)
"""Trainium2 Bass kernel for nn_Algebraic: out = [x, all 2-subset col products,
all 3-subset col products] for x of shape [262144, 16] fp32.

Output layout (matches itertools.combinations lexicographic order):
  cols [0,16)     : x itself
  cols [16,136)   : pairs (a,b), a<b, lexicographic
  cols [136,696)  : triples (a,b,c), a<b<c, lexicographic

Key structure exploited: for a fixed smallest index a, the triple block
(a, b, c) is contiguous in the output AND equals x[:, a] * (the contiguous
tail of the pair block consisting of pairs (b, c) with b > a). So the whole
output needs only 15 + 14 = 29 broadcast tensor_tensor multiplies per tile.

Raw Bass (not Tile): the walrus codegen here supports at most one semaphore
wait per instruction, and Tile's non-transitively-minimal wait placement
emits two on pipelined DMAs. With standalone wait_ge instructions and a
hand-rolled double buffer we stay within the limit by construction.

Per supertile (2048 rows): partition p holds 16 consecutive rows, so both
the input DMA (128 KB) and the output DMA (5.7 MB) are fully contiguous per
partition. Engines: ACT issues input DMAs, SP issues output DMAs, VectorE
does all compute. Double-buffered x/out tiles.

Sharding: data-parallel over batch: 262144 rows / 8 cores = 32768 rows/core.
Each core runs the same NEFF (SPMD) on its row shard.
"""

import numpy as np

N = 16           # input columns
N_PAIRS = 120    # C(16,2)
N_TRIPLES = 560  # C(16,3)
OUT_COLS = N + N_PAIRS + N_TRIPLES  # 696
P = 128          # SBUF partitions

BATCH = 262144
N_CORES = 8
ROWS_PER_CORE = BATCH // N_CORES  # 32768

T = 16           # rows per partition per supertile
SUP = ROWS_PER_CORE // (P * T)  # 16 supertiles per core

# pstart[a]: index within the pair block where pairs with first elem >= a start
_pstart = [0]
for _a in range(N):
    _pstart.append(_pstart[-1] + (N - 1 - _a))
POFF = [N + _pstart[a] for a in range(N)]  # column where pair group a starts


def _c2(n):
    return n * (n - 1) // 2


_tstart = [0]
for _a in range(N):
    _tstart.append(_tstart[-1] + _c2(N - 1 - _a))
TOFF = [N + N_PAIRS + _tstart[a] for a in range(N)]  # triple group a start


def _compute_supertile(nc, vector, x_sb, o_sb, t, s_pair, pair_tick):
    """Emit the VectorE ops for one supertile; returns the last op.

    s_pair/pair_tick: explicit handshake between the last pair op and the
    first triple op (triples read the pair block). Hardware serializes DVE
    ops anyway; the race detector wants the edge explicit.
    """
    xr = x_sb.ap()[:, :t * N].rearrange("p (t c) -> p t c", c=N)
    outr = o_sb.ap()[:, :t * OUT_COLS].rearrange("p (t c) -> p t c", c=OUT_COLS)

    nc.vector.tensor_copy(out=outr[:, :, 0:N], in_=xr[:, :, :])
    for a in range(N - 1):          # pairs: x[:,a] * x[:,a+1:16]
        ln = N - 1 - a
        op = nc.vector.tensor_mul(
            out=outr[:, :, POFF[a]:POFF[a] + ln],
            in0=xr[:, :, a:a + 1].to_broadcast([P, t, ln]),
            in1=xr[:, :, a + 1:N],
        )
        if a == N - 2:
            op.then_inc(s_pair, 1)
    vector.wait_ge(s_pair, pair_tick)
    last = None
    for a in range(N - 2):          # triples: x[:,a] * pairs[(b,c): b>a]
        ln = _c2(N - 1 - a)
        tail = N + _pstart[a + 1]
        last = nc.vector.tensor_mul(
            out=outr[:, :, TOFF[a]:TOFF[a] + ln],
            in0=xr[:, :, a:a + 1].to_broadcast([P, t, ln]),
            in1=outr[:, :, tail:tail + ln],
        )
    return last


def build_nc(rows_per_core=ROWS_PER_CORE, t=T, reps=1, ramp=True):
    """reps > 1 repeats the whole pipeline (idempotent rewrites of the same
    output) — used only for timing calibration, never for grading.

    ramp=True splits the first supertile into 4 quarter chunks so the first
    output DMA starts ~4x earlier (HBM is idle during the first compute)."""
    import concourse.bass as bass
    import concourse.mybir as mybir

    sup = rows_per_core // (P * t)
    assert sup * P * t == rows_per_core

    if ramp and sup >= 2 and t % 4 == 0:
        ts = [t // 4] * 4 + [t] * (sup - 1)
    else:
        ts = [t] * sup
    nchunks = len(ts)
    starts = [0]
    for ti in ts:
        starts.append(starts[-1] + P * ti)
    assert starts[-1] == rows_per_core

    nc = bass.Bass(trn_type="TRN2")
    x = nc.dram_tensor("x", [rows_per_core, N], mybir.dt.float32,
                       kind="ExternalInput")
    y = nc.dram_tensor("y", [rows_per_core, OUT_COLS], mybir.dt.float32,
                       kind="ExternalOutput")

    # chunk i covers rows [starts[i], starts[i+1]); partition p holds ts[i]
    # consecutive rows: fully contiguous per-partition DMAs on both sides.
    def xv(i):
        r0, r1 = starts[i], starts[i + 1]
        return x.ap()[r0:r1, :].rearrange("(p t) c -> p (t c)", p=P)

    def yv(i):
        r0, r1 = starts[i], starts[i + 1]
        return y.ap()[r0:r1, :].rearrange("(p t) c -> p (t c)", p=P)

    # 3 output slots decouple compute(s) from the drain of out(s-2): with
    # only 2, each slot alternates compute / DMA and both DMA rings idle
    # half the time. 2 input slots suffice (inputs are tiny).
    NXB, NOB = 2, 3
    x_sb = [nc.alloc_sbuf_tensor(f"x_sb{i}", [P, t * N], mybir.dt.float32)
            for i in range(NXB)]
    o_sb = [nc.alloc_sbuf_tensor(f"o_sb{i}", [P, t * OUT_COLS],
                                 mybir.dt.float32) for i in range(NOB)]

    # Every semaphore has a single incrementing engine and strictly ordered
    # increments (sem-gated), so ">= 16k" waits are unambiguous.
    # out(j) runs on ring j%2 and reads slot j%3; sem index j%6 gives each
    # (slot, ring) pair its own counter.
    s_in = [nc.alloc_semaphore(f"s_in{i}") for i in range(NXB)]
    s_out = [nc.alloc_semaphore(f"s_out{i}") for i in range(6)]
    s_cmp = nc.alloc_semaphore("s_cmp")  # completed compute supertiles (+1)
    s_pair = nc.alloc_semaphore("s_pair")  # pairs-done ticks (+1 per supertile)

    with nc.Block() as block:

        nsup = nchunks * reps

        def chunk_t(s):
            return ts[s % nchunks]

        @block.gpsimd
        def _(gpsimd):
            # input DMAs via SWDGE, keeping both HWDGE rings free for output
            for s in range(nsup):
                if s >= NXB:
                    # compute(s-NXB) done => x slot free for reuse
                    gpsimd.wait_ge(s_cmp, s - NXB + 1)
                ti = chunk_t(s)
                gpsimd.dma_start(
                    out=x_sb[s % NXB].ap()[:, :ti * N],
                    in_=xv(s % nchunks)).then_inc(s_in[s % NXB], 16)

        @block.vector
        def _(vector):
            for s in range(nsup):
                vector.wait_ge(s_in[s % NXB], 16 * (s // NXB + 1))
                if s >= NOB:
                    # out(s-NOB) flushed => out slot free for overwrite
                    j = s - NOB
                    vector.wait_ge(s_out[j % 6], 16 * (j // 6 + 1))
                _compute_supertile(nc, vector, x_sb[s % NXB], o_sb[s % NOB],
                                   chunk_t(s), s_pair, s + 1).then_inc(s_cmp, 1)

        # Output DMAs alternate between the two HWDGE rings (SP for even
        # supertiles, ACT for odd) so both rings stream concurrently and
        # the ~2us per-DMA fixed completion latency overlaps.
        @block.sync
        def _(sync):
            for s in range(0, nsup, 2):
                sync.wait_ge(s_cmp, s + 1)
                ti = chunk_t(s)
                sync.dma_start(
                    out=yv(s % nchunks),
                    in_=o_sb[s % NOB].ap()[:, :ti * OUT_COLS],
                ).then_inc(s_out[s % 6], 16)

        @block.scalar
        def _(scalar):
            for s in range(1, nsup, 2):
                scalar.wait_ge(s_cmp, s + 1)
                ti = chunk_t(s)
                scalar.dma_start(
                    out=yv(s % nchunks),
                    in_=o_sb[s % NOB].ap()[:, :ti * OUT_COLS],
                ).then_inc(s_out[s % 6], 16)

    return nc


_CACHED = {}


def _get_nc():
    key = (ROWS_PER_CORE, T)
    if key not in _CACHED:
        _CACHED[key] = build_nc()
    return _CACHED[key]


def kernel(x):
    from concourse.bass_utils import run_bass_kernel_spmd

    x = np.asarray(x, dtype=np.float32)
    assert x.shape == (BATCH, N), x.shape
    nc = _get_nc()
    in_maps = [
        {"x": np.ascontiguousarray(x[c * ROWS_PER_CORE:(c + 1) * ROWS_PER_CORE])}
        for c in range(N_CORES)
    ]
    res = run_bass_kernel_spmd(nc, in_maps, core_ids=list(range(N_CORES)))
    return np.concatenate([r["y"] for r in res.results], axis=0)


# revision 18
# speedup vs baseline: 1.4016x; 1.4016x over previous
"""Trainium2 Bass kernel for nn_Algebraic: out = [x, all 2-subset col products,
all 3-subset col products] for x of shape [262144, 16] fp32.

Output layout (matches itertools.combinations lexicographic order):
  cols [0,16)     : x itself
  cols [16,136)   : pairs (a,b), a<b, lexicographic
  cols [136,696)  : triples (a,b,c), a<b<c, lexicographic

Key structure exploited: for a fixed smallest index a, the triple block
(a, b, c) is contiguous in the output AND equals x[:, a] * (the contiguous
tail of the pair block consisting of pairs (b, c) with b > a). So the whole
output needs only 15 + 14 = 29 broadcast tensor_tensor multiplies per tile.

Raw Bass (not Tile): the walrus codegen here supports at most one semaphore
wait per instruction, and Tile's non-transitively-minimal wait placement
emits two on pipelined DMAs. With standalone wait_ge instructions and a
hand-rolled double buffer we stay within the limit by construction.

Per supertile (2048 rows): partition p holds 16 consecutive rows, so both
the input DMA (128 KB) and the output DMA (5.7 MB) are fully contiguous per
partition. Engines: ACT issues input DMAs, SP issues output DMAs, VectorE
does all compute. Double-buffered x/out tiles.

Sharding: data-parallel over batch: 262144 rows / 8 cores = 32768 rows/core.
Each core runs the same NEFF (SPMD) on its row shard.
"""

import numpy as np

N = 16           # input columns
N_PAIRS = 120    # C(16,2)
N_TRIPLES = 560  # C(16,3)
OUT_COLS = N + N_PAIRS + N_TRIPLES  # 696
P = 128          # SBUF partitions

BATCH = 262144
N_CORES = 8
ROWS_PER_CORE = BATCH // N_CORES  # 32768

T = 16           # rows per partition per supertile
SUP = ROWS_PER_CORE // (P * T)  # 16 supertiles per core

# pstart[a]: index within the pair block where pairs with first elem >= a start
_pstart = [0]
for _a in range(N):
    _pstart.append(_pstart[-1] + (N - 1 - _a))
POFF = [N + _pstart[a] for a in range(N)]  # column where pair group a starts


def _c2(n):
    return n * (n - 1) // 2


_tstart = [0]
for _a in range(N):
    _tstart.append(_tstart[-1] + _c2(N - 1 - _a))
TOFF = [N + N_PAIRS + _tstart[a] for a in range(N)]  # triple group a start


# Number of leading (largest) triple groups computed on GPSIMD instead of
# VectorE. Real DVE tensor_tensor overhead (~151 cyc/op measured vs 58
# modeled) makes DVE the hardware critical path otherwise; GPSIMD is idle.
GP_TRI = 3


def _tile_views(x_sb, o_sb, t):
    xr = x_sb.ap()[:, :t * N].rearrange("p (t c) -> p t c", c=N)
    outr = o_sb.ap()[:, :t * OUT_COLS].rearrange("p (t c) -> p t c",
                                                 c=OUT_COLS)
    return xr, outr


def _triple_op(eng, xr, outr, t, a):
    ln = _c2(N - 1 - a)
    tail = N + _pstart[a + 1]
    return eng.tensor_mul(
        out=outr[:, :, TOFF[a]:TOFF[a] + ln],
        in0=xr[:, :, a:a + 1].to_broadcast([P, t, ln]),
        in1=outr[:, :, tail:tail + ln],
    )


def _compute_supertile(nc, vector, x_sb, o_sb, t, s_pair, pair_tick):
    """Emit the VectorE ops for one supertile; returns the last op.

    s_pair/pair_tick: explicit handshake between the last pair op and the
    first triple op (triples read the pair block). Hardware serializes DVE
    ops anyway; the race detector wants the edge explicit.
    """
    xr, outr = _tile_views(x_sb, o_sb, t)

    nc.vector.tensor_copy(out=outr[:, :, 0:N], in_=xr[:, :, :])
    for a in range(N - 1):          # pairs: x[:,a] * x[:,a+1:16]
        ln = N - 1 - a
        op = nc.vector.tensor_mul(
            out=outr[:, :, POFF[a]:POFF[a] + ln],
            in0=xr[:, :, a:a + 1].to_broadcast([P, t, ln]),
            in1=xr[:, :, a + 1:N],
        )
        if a == N - 2:
            op.then_inc(s_pair, 1)
    vector.wait_ge(s_pair, pair_tick)
    last = None
    for a in range(GP_TRI, N - 2):  # triples: x[:,a] * pairs[(b,c): b>a]
        last = _triple_op(nc.vector, xr, outr, t, a)
    return last


def build_nc(rows_per_core=ROWS_PER_CORE, t=T, reps=1, ramp=True):
    """reps > 1 repeats the whole pipeline (idempotent rewrites of the same
    output) — used only for timing calibration, never for grading.

    ramp=True splits the first supertile into 4 quarter chunks so the first
    output DMA starts ~4x earlier (HBM is idle during the first compute)."""
    import concourse.bass as bass
    import concourse.mybir as mybir

    sup = rows_per_core // (P * t)
    assert sup * P * t == rows_per_core

    if ramp and sup >= 2 and t % 4 == 0:
        ts = [t // 4] * 4 + [t] * (sup - 1)
    else:
        ts = [t] * sup
    nchunks = len(ts)
    starts = [0]
    for ti in ts:
        starts.append(starts[-1] + P * ti)
    assert starts[-1] == rows_per_core

    nc = bass.Bass(trn_type="TRN2")
    x = nc.dram_tensor("x", [rows_per_core, N], mybir.dt.float32,
                       kind="ExternalInput")
    y = nc.dram_tensor("y", [rows_per_core, OUT_COLS], mybir.dt.float32,
                       kind="ExternalOutput")

    # chunk i covers rows [starts[i], starts[i+1]); partition p holds ts[i]
    # consecutive rows: fully contiguous per-partition DMAs on both sides.
    def xv(i):
        r0, r1 = starts[i], starts[i + 1]
        return x.ap()[r0:r1, :].rearrange("(p t) c -> p (t c)", p=P)

    def yv(i):
        r0, r1 = starts[i], starts[i + 1]
        return y.ap()[r0:r1, :].rearrange("(p t) c -> p (t c)", p=P)

    # 3 output slots decouple compute(s) from the drain of out(s-2): with
    # only 2, each slot alternates compute / DMA and both DMA rings idle
    # half the time. 2 input slots suffice (inputs are tiny).
    NXB, NOB = 2, 3
    x_sb = [nc.alloc_sbuf_tensor(f"x_sb{i}", [P, t * N], mybir.dt.float32)
            for i in range(NXB)]
    o_sb = [nc.alloc_sbuf_tensor(f"o_sb{i}", [P, t * OUT_COLS],
                                 mybir.dt.float32) for i in range(NOB)]

    # Every semaphore has a single incrementing engine and strictly ordered
    # increments (sem-gated), so ">= 16k" waits are unambiguous.
    # out(j) runs on ring j%2 and reads slot j%3; sem index j%6 gives each
    # (slot, ring) pair its own counter.
    s_in = [nc.alloc_semaphore(f"s_in{i}") for i in range(NXB)]
    s_out = [nc.alloc_semaphore(f"s_out{i}") for i in range(6)]
    s_cmp = nc.alloc_semaphore("s_cmp")  # completed DVE supertiles (+1)
    s_pair = nc.alloc_semaphore("s_pair")  # pairs-done ticks (+1 per supertile)
    s_gp = nc.alloc_semaphore("s_gp")  # completed GPSIMD supertiles (+1)

    with nc.Block() as block:

        nsup = nchunks * reps

        def chunk_t(s):
            return ts[s % nchunks]

        def dma_in(gpsimd, s):
            ti = chunk_t(s)
            gpsimd.dma_start(
                out=x_sb[s % NXB].ap()[:, :ti * N],
                in_=xv(s % nchunks)).then_inc(s_in[s % NXB], 16)

        @block.gpsimd
        def _(gpsimd):
            # GPSIMD: input DMAs via SWDGE (keeps both HWDGE rings free for
            # output) + the GP_TRI largest triple groups per supertile.
            for s in range(min(NXB, nsup)):
                dma_in(gpsimd, s)
            for s in range(nsup):
                # pairs(s) on DVE done => pair block readable
                gpsimd.wait_ge(s_pair, s + 1)
                ti = chunk_t(s)
                xr, outr = _tile_views(x_sb[s % NXB], o_sb[s % NOB], ti)
                op = None
                for a in range(GP_TRI):
                    op = _triple_op(nc.gpsimd, xr, outr, ti, a)
                op.then_inc(s_gp, 1)
                if s + NXB < nsup:
                    # DVE compute(s) done => x slot free for reuse. Own
                    # reads of slot s%NXB precede this in program order;
                    # the s_gp wait makes that edge explicit for the
                    # async-DMA race check.
                    gpsimd.wait_ge(s_cmp, s + 1)
                    gpsimd.wait_ge(s_gp, s + 1)
                    dma_in(gpsimd, s + NXB)

        @block.vector
        def _(vector):
            for s in range(nsup):
                vector.wait_ge(s_in[s % NXB], 16 * (s // NXB + 1))
                if s >= NOB:
                    # out(s-NOB) flushed => out slot free for overwrite
                    j = s - NOB
                    vector.wait_ge(s_out[j % 6], 16 * (j // 6 + 1))
                _compute_supertile(nc, vector, x_sb[s % NXB], o_sb[s % NOB],
                                   chunk_t(s), s_pair, s + 1).then_inc(s_cmp, 1)

        # Output DMAs alternate between the two HWDGE rings (SP for even
        # supertiles, ACT for odd) so both rings stream concurrently and
        # the ~2us per-DMA fixed completion latency overlaps.
        @block.sync
        def _(sync):
            for s in range(0, nsup, 2):
                sync.wait_ge(s_cmp, s + 1)
                sync.wait_ge(s_gp, s + 1)
                ti = chunk_t(s)
                sync.dma_start(
                    out=yv(s % nchunks),
                    in_=o_sb[s % NOB].ap()[:, :ti * OUT_COLS],
                ).then_inc(s_out[s % 6], 16)

        @block.scalar
        def _(scalar):
            for s in range(1, nsup, 2):
                scalar.wait_ge(s_cmp, s + 1)
                scalar.wait_ge(s_gp, s + 1)
                ti = chunk_t(s)
                scalar.dma_start(
                    out=yv(s % nchunks),
                    in_=o_sb[s % NOB].ap()[:, :ti * OUT_COLS],
                ).then_inc(s_out[s % 6], 16)

    return nc


_CACHED = {}


def _get_nc():
    key = (ROWS_PER_CORE, T)
    if key not in _CACHED:
        _CACHED[key] = build_nc()
    return _CACHED[key]


def kernel(x):
    from concourse.bass_utils import run_bass_kernel_spmd

    x = np.asarray(x, dtype=np.float32)
    assert x.shape == (BATCH, N), x.shape
    nc = _get_nc()
    in_maps = [
        {"x": np.ascontiguousarray(x[c * ROWS_PER_CORE:(c + 1) * ROWS_PER_CORE])}
        for c in range(N_CORES)
    ]
    res = run_bass_kernel_spmd(nc, in_maps, core_ids=list(range(N_CORES)))
    return np.concatenate([r["y"] for r in res.results], axis=0)
